# revision 16
# baseline (speedup 1.0000x reference)
"""TRN2 Bass kernel for nn_DCM_50414326120808 (dense_cnn).

Computes, for x, convoluted [16, 256, 96, 96]:
  pooled = adaptive_avg_pool2d(x, 3)                         # [16,256,3,3]
  gen    = 1x1 conv (w_gen) of pooled + b_gen                # per-sample filters
  y      = conv3x3(convoluted, w_c1) + b_c1                  # [16,256,96,96]
  y      = relu(batchnorm_train(y) * gamma + beta)
  out    = depthwise 3x3 conv of y with per-(sample,channel) filters gen

Wall-clock here is dominated by the axon host<->device wire (~40-55 MB/s),
not device compute, so the design minimizes wire bytes:
 - x is never sent: pooling + the 1x1 filter generation are exact f32 host
   math (one cheap reduction pass + a tiny matmul).
 - convoluted crosses the wire in fp16 (75.5 MB instead of 151), unpadded;
   zero-padding happens on device in SBUF.
 - the output crosses back in fp16 and is upcast to f32 on host.
 - b_c1 is dropped entirely: training-mode BN subtracts the per-channel
   mean, so a constant per-channel bias cancels exactly.
 - the donated output buffers are created device-side (jnp.zeros under jit)
   instead of shipping host zeros through the tunnel.
 - a custom shard_map runner feeds one globally-sharded array per input
   (batch-sharded over 8 cores), avoiding the per-core dict -> concat copies
   of run_bass_kernel_spmd.

Device mapping (per core, 2 samples):
 - conv3x3 -> 18 accumulated TensorE matmuls (9 taps x 2 input-channel
   chunks) per 4-row output tile, fp16 operands, f32 PSUM accumulate.
 - BN stats via DVE bn_stats on each conv PSUM tile + bn_aggr + AllReduce.
 - depthwise conv -> 9 accumulated matmuls with diagonal weight matrices
   diag(gen[:, tap]) built on DVE from an identity matrix.
"""

import hashlib

import numpy as np

import jax
import jax.numpy as jnp
from jax.sharding import Mesh, NamedSharding, PartitionSpec
from jax.experimental.shard_map import shard_map

try:
    jax.config.update("jax_compilation_cache_dir", "/tmp/jax_cache_nn_dcm")
    jax.config.update("jax_persistent_cache_min_entry_size_bytes", -1)
    jax.config.update("jax_persistent_cache_min_compile_time_secs", 0.0)
except Exception:
    pass

import concourse.bass as bass  # noqa: F401  (registers bass machinery)
import concourse.bacc as bacc
import concourse.tile as tile
from concourse import mybir
from concourse.bass2jax import (
    _bass_exec_p,
    install_neuronx_cc_hook,
    partition_id_tensor,
)

F32 = mybir.dt.float32
F16 = mybir.dt.float16

B, C, H, W = 16, 256, 96, 96
FS = 3
BN_EPS = 1e-5
NCORES = 8
SPC = B // NCORES          # samples per core = 2
P = 128                    # partition dim
NIC = C // P               # input channel chunks = 2
NOC = C // P               # output channel chunks = 2
HP, WP = H + 2, W + 2      # padded spatial = 98
RT = 4                     # output rows per tile
NT = H // RT               # tiles per (sample, oc) = 24
GRP = 6                    # tiles per input group (24 rows)
NG = NT // GRP             # input groups = 4
GR = GRP * RT              # rows per input group = 24
N_LOCAL = float(SPC * H * W)        # elements per (channel, core)
N_TOTAL = float(B * H * W)          # elements per channel globally

_cache = {}


def _build_program():
    nc = bacc.Bacc("TRN2", target_bir_lowering=False, debug=False,
                   num_devices=NCORES)

    cv_d = nc.dram_tensor("cv", (SPC, NIC, P, H, W), F16, kind="ExternalInput")
    gen_d = nc.dram_tensor("genf", (SPC, NOC, P, 9), F32, kind="ExternalInput")
    wT_d = nc.dram_tensor("wT", (NIC, P, 9 * NOC * P), F16, kind="ExternalInput")
    gam_d = nc.dram_tensor("gam", (NOC, P), F32, kind="ExternalInput")
    bet_d = nc.dram_tensor("bet", (NOC, P), F32, kind="ExternalInput")
    id_d = nc.dram_tensor("ident", (P, P), F16, kind="ExternalInput")
    out_d = nc.dram_tensor("out", (SPC, NOC, P, H, W), F16, kind="ExternalOutput")

    with tile.TileContext(nc) as tc:
        with (
            tc.tile_pool(name="const", bufs=1) as const,
            tc.tile_pool(name="cin", bufs=4) as cinp,
            tc.tile_pool(name="small", bufs=1) as small,
            tc.tile_pool(name="ybn", bufs=1) as ybnp,
            tc.tile_pool(name="yld", bufs=3) as yldp,
            tc.tile_pool(name="evac", bufs=4) as evacp,
            tc.tile_pool(name="diag", bufs=2) as diagp,
            tc.tile_pool(name="ps_conv", bufs=3, space="PSUM") as ps_conv,
            tc.tile_pool(name="ps_dw", bufs=3, space="PSUM") as ps_dw,
            tc.tile_pool(name="dram", bufs=1, space="DRAM") as dram,
        ):
            # ---- constants / weights ----
            w_sb = const.tile([P, NIC, 9 * NOC * P], F16)
            for ic in range(NIC):
                nc.sync.dma_start(w_sb[:, ic, :], wT_d.ap()[ic])
            id_sb = const.tile([P, P], F16)
            nc.sync.dma_start(id_sb[:], id_d.ap())
            gam_sb = const.tile([P, NOC], F32)
            bet_sb = const.tile([P, NOC], F32)
            nc.sync.dma_start(gam_sb[:], gam_d.ap().rearrange("a p -> p a"))
            nc.sync.dma_start(bet_sb[:], bet_d.ap().rearrange("a p -> p a"))
            gen = {}
            for s in range(SPC):
                for oc in range(NOC):
                    gt = small.tile([P, 9], F32, tag=f"gen{s}{oc}",
                                    name=f"gen{s}{oc}")
                    gen[s, oc] = gt
                    nc.sync.dma_start(gt[:], gen_d.ap()[s, oc])

            y_spill = dram.tile([SPC, NOC, P, H, W], F16)
            ar_in_d = dram.tile([P, 2 * NOC], F32)
            ar_out_d = dram.tile([P, 2 * NOC], F32)

            # ---- conv3x3 (device-side zero pad) + BN stats + spill ----
            stats = small.tile([P, NOC, NT * SPC * 6], F32)
            for s in range(SPC):
                for g in range(NG):
                    cin = {}
                    for ic in range(NIC):
                        ct = cinp.tile([P, GR + 2, WP], F16, name="cin")
                        cin[ic] = ct
                        # zero the pad borders, then DMA the valid interior
                        nc.gpsimd.memset(ct[:, :, 0:1], 0)
                        nc.gpsimd.memset(ct[:, :, WP - 1:WP], 0)
                        if g == 0:
                            nc.gpsimd.memset(ct[:, 0, :], 0)
                            nc.sync.dma_start(
                                ct[:, 1:GR + 2, 1:W + 1],
                                cv_d.ap()[s, ic, :, 0:GR + 1, :])
                        elif g == NG - 1:
                            nc.gpsimd.memset(ct[:, GR + 1, :], 0)
                            nc.sync.dma_start(
                                ct[:, 0:GR + 1, 1:W + 1],
                                cv_d.ap()[s, ic, :, g * GR - 1:H, :])
                        else:
                            nc.sync.dma_start(
                                ct[:, :, 1:W + 1],
                                cv_d.ap()[s, ic, :, g * GR - 1:g * GR + GR + 1, :])
                    for jj in range(GRP):
                        j = g * GRP + jj
                        for oc in range(NOC):
                            ps = ps_conv.tile([P, RT, W], F32, name="ps")
                            k = 0
                            for ic in range(NIC):
                                for t in range(9):
                                    dy, dx = t // 3, t % 3
                                    r0 = jj * RT + dy
                                    nc.tensor.matmul(
                                        ps[:],
                                        w_sb[:, ic, (t * NOC + oc) * P:
                                             (t * NOC + oc + 1) * P],
                                        cin[ic][:, r0:r0 + RT, dx:dx + W],
                                        start=(k == 0), stop=(k == 17))
                                    k += 1
                            idx = (s * NT + j) * 6
                            nc.vector.bn_stats(
                                stats[:, oc, idx:idx + 6],
                                ps[:].rearrange("p a b -> p (a b)"))
                            ysb = evacp.tile([P, RT, W], F16, name="ysb")
                            nc.scalar.copy(ysb[:], ps[:])
                            nc.sync.dma_start(
                                y_spill[s, oc, :, j * RT:(j + 1) * RT, :], ysb[:])

            # ---- merge stats, AllReduce, compute scale/bias ----
            ar_in = small.tile([P, 2 * NOC], F32)
            mvt = small.tile([P, NOC, 2], F32)
            tmp = small.tile([P, 4], F32)
            for oc in range(NOC):
                nc.vector.bn_aggr(mvt[:, oc, :], stats[:, oc, :])
                # sum = n * mean ; sumsq = n * (var + mean^2)
                nc.vector.tensor_scalar_mul(ar_in[:, 2 * oc:2 * oc + 1],
                                            mvt[:, oc, 0:1], N_LOCAL)
                nc.vector.tensor_mul(tmp[:, 0:1], mvt[:, oc, 0:1], mvt[:, oc, 0:1])
                nc.vector.tensor_add(tmp[:, 1:2], tmp[:, 0:1], mvt[:, oc, 1:2])
                nc.vector.tensor_scalar_mul(ar_in[:, 2 * oc + 1:2 * oc + 2],
                                            tmp[:, 1:2], N_LOCAL)
            nc.sync.dma_start(ar_in_d[:], ar_in[:])
            nc.gpsimd.collective_compute(
                "AllReduce", mybir.AluOpType.add,
                replica_groups=[list(range(NCORES))],
                ins=[ar_in_d.opt()], outs=[ar_out_d.opt()])
            ar_out = small.tile([P, 2 * NOC], F32)
            nc.sync.dma_start(ar_out[:], ar_out_d[:])

            scale = small.tile([P, NOC], F32)
            bias = small.tile([P, NOC], F32)
            w1 = small.tile([P, 8], F32)
            for oc in range(NOC):
                mu = w1[:, 0:1]
                veps = w1[:, 1:2]
                nc.vector.tensor_scalar_mul(mu, ar_out[:, 2 * oc:2 * oc + 1],
                                            1.0 / N_TOTAL)
                # var = sumsq/n - mu^2 ; veps = var + eps
                nc.vector.tensor_scalar_mul(w1[:, 2:3],
                                            ar_out[:, 2 * oc + 1:2 * oc + 2],
                                            1.0 / N_TOTAL)
                nc.vector.tensor_mul(w1[:, 3:4], mu, mu)
                nc.vector.tensor_sub(w1[:, 4:5], w1[:, 2:3], w1[:, 3:4])
                nc.vector.tensor_scalar_add(veps, w1[:, 4:5], BN_EPS)
                # r = rsqrt(veps): reciprocal + ACT sqrt + one Newton step
                inv = w1[:, 5:6]
                nc.vector.reciprocal(inv, veps)
                r = w1[:, 6:7]
                nc.scalar.activation(r, inv, mybir.ActivationFunctionType.Sqrt)
                # r <- 0.5 * r * (3 - veps * r^2)
                nc.vector.tensor_mul(w1[:, 7:8], r, r)
                nc.vector.tensor_mul(w1[:, 7:8], w1[:, 7:8], veps)
                nc.vector.tensor_scalar(w1[:, 7:8], w1[:, 7:8], -0.5, 1.5,
                                        op0=mybir.AluOpType.mult,
                                        op1=mybir.AluOpType.add)
                nc.vector.tensor_mul(r, r, w1[:, 7:8])
                # scale = gamma * r ; bias = beta - mu * scale
                nc.vector.tensor_mul(scale[:, oc:oc + 1], gam_sb[:, oc:oc + 1], r)
                nc.vector.tensor_mul(w1[:, 7:8], mu, scale[:, oc:oc + 1])
                nc.vector.tensor_sub(bias[:, oc:oc + 1], bet_sb[:, oc:oc + 1],
                                     w1[:, 7:8])

            # ---- BN apply + ReLU + dynamic depthwise conv ----
            for s in range(SPC):
                for oc in range(NOC):
                    dg = diagp.tile([P, 9, P], F16, name="dg")
                    for t in range(9):
                        nc.vector.tensor_scalar_mul(dg[:, t, :], id_sb[:],
                                                    gen[s, oc][:, t:t + 1])
                    ybn = ybnp.tile([P, HP, WP], F16, name="ybn")
                    nc.gpsimd.memset(ybn[:, 0, :], 0)
                    nc.gpsimd.memset(ybn[:, HP - 1, :], 0)
                    # interior edge pads: (r, 97) and (r+1, 0) are flat-adjacent
                    pad_pairs = (ybn[:].rearrange("p a b -> p (a b)")
                                 [:, WP - 1:WP - 1 + H * WP]
                                 .rearrange("p (r t) -> p r t", t=WP)[:, :, 0:2])
                    nc.gpsimd.memset(pad_pairs, 0)
                    RB = 24
                    for rb in range(H // RB):
                        yld = yldp.tile([P, RB, W], F16, name="yld")
                        nc.sync.dma_start(
                            yld[:], y_spill[s, oc, :, rb * RB:(rb + 1) * RB, :])
                        nc.scalar.activation(
                            ybn[:, 1 + rb * RB:1 + (rb + 1) * RB, 1:W + 1],
                            yld[:], mybir.ActivationFunctionType.Relu,
                            bias=bias[:, oc:oc + 1], scale=scale[:, oc:oc + 1])
                    for j in range(NT):
                        pd = ps_dw.tile([P, RT, W], F32, name="pd")
                        for t in range(9):
                            dy, dx = t // 3, t % 3
                            nc.tensor.matmul(
                                pd[:], dg[:, t, :],
                                ybn[:, j * RT + dy:j * RT + dy + RT, dx:dx + W],
                                start=(t == 0), stop=(t == 8))
                        osb = evacp.tile([P, RT, W], F16, name="osb")
                        nc.vector.tensor_copy(osb[:], pd[:])
                        nc.sync.dma_start(
                            out_d.ap()[s, oc, :, j * RT:(j + 1) * RT, :], osb[:])

    nc.compile()
    return nc


def _build_runner():
    nc = _build_program()
    install_neuronx_cc_hook()

    partition_name = (nc.partition_id_tensor.name
                      if nc.partition_id_tensor else None)
    in_names = []
    out_names = []
    out_avals = []
    for alloc in nc.m.functions[0].allocations:
        if not isinstance(alloc, mybir.MemoryLocationSet):
            continue
        name = alloc.memorylocations[0].name
        if alloc.kind == "ExternalInput":
            if name != partition_name:
                in_names.append(name)
        elif alloc.kind == "ExternalOutput":
            assert alloc.tensor_shape is not None and alloc.dtype is not None
            out_names.append(name)
            out_avals.append(jax.core.ShapedArray(
                tuple(alloc.tensor_shape), mybir.dt.np(alloc.dtype)))
    n_params = len(in_names)
    n_outs = len(out_avals)
    all_in_names = tuple(in_names) + tuple(out_names)
    if partition_name is not None:
        all_in_names = all_in_names + (partition_name,)

    def _body(*args):
        operands = list(args)
        if partition_name is not None:
            operands.append(partition_id_tensor())
        outs = _bass_exec_p.bind(
            *operands,
            out_avals=tuple(out_avals),
            in_names=all_in_names,
            out_names=tuple(out_names),
            lowering_input_output_aliases=(),
            sim_require_finite=True,
            sim_require_nnan=True,
            nc=nc,
        )
        return tuple(outs)

    devices = jax.devices()[:NCORES]
    mesh = Mesh(np.asarray(devices), ("core",))
    spec = PartitionSpec("core")
    jitted = jax.jit(
        shard_map(_body, mesh=mesh,
                  in_specs=(spec,) * (n_params + n_outs),
                  out_specs=(spec,) * n_outs,
                  check_rep=False),
        donate_argnums=tuple(range(n_params, n_params + n_outs)),
        keep_unused=True,
    )
    zero_shapes = [(NCORES * a.shape[0], *a.shape[1:]) for a in out_avals]
    zero_dtypes = [a.dtype for a in out_avals]
    zjit = jax.jit(
        lambda: tuple(jnp.zeros(s, d) for s, d in zip(zero_shapes, zero_dtypes)),
        out_shardings=tuple(NamedSharding(mesh, spec) for _ in out_avals),
    )
    return {"nc": nc, "jitted": jitted, "zjit": zjit,
            "in_names": in_names, "out_names": out_names,
            "mesh": mesh, "devices": devices,
            "sharding": NamedSharding(mesh, spec)}


def _pool9(x):
    """Exact adaptive_avg_pool2d(x, 3) sums, flattened: [B, C, 9] (f32 sums)."""
    x = np.asarray(x, np.float32)
    r = x.reshape(B * C, H, 3, W // 3).sum(axis=3)
    p = r.reshape(B * C, 3, H // 3, 3).sum(axis=2)
    return p.reshape(B, C, 9)


def _device_weights(r, w_c1, gamma, beta):
    """Transfer the (replicated) conv/BN params once; reuse across calls."""
    w_c1 = np.asarray(w_c1, np.float32)
    gamma = np.asarray(gamma, np.float32)
    beta = np.asarray(beta, np.float32)
    key = hashlib.blake2b(
        w_c1.tobytes() + gamma.tobytes() + beta.tobytes(), digest_size=16
    ).digest()
    if _cache.get("wkey") == key:
        return _cache["wdev"]
    # wT[ic, i, ((t*NOC)+oc)*P+o] = w_c1[oc*P+o, ic*P+i, dy, dx]
    wT = np.ascontiguousarray(
        w_c1.reshape(NOC, P, NIC, P, 9).transpose(2, 3, 4, 0, 1)
    ).reshape(NIC, P, 9 * NOC * P).astype(np.float16)
    sh = r["sharding"]
    wdev = {
        "wT": jax.device_put(np.tile(wT, (NCORES, 1, 1)), sh),
        "gam": jax.device_put(np.tile(gamma.reshape(NOC, P), (NCORES, 1)), sh),
        "bet": jax.device_put(np.tile(beta.reshape(NOC, P), (NCORES, 1)), sh),
        "ident": jax.device_put(
            np.tile(np.eye(P, dtype=np.float16), (NCORES, 1)), sh),
    }
    _cache["wkey"] = key
    _cache["wdev"] = wdev
    return wdev


def kernel(**inputs) -> np.ndarray:
    import os
    import time
    ktime = os.environ.get("KTIME", "0") == "1"
    t0 = time.time()

    if "runner" not in _cache:
        _cache["runner"] = _build_runner()
    r = _cache["runner"]

    x = np.asarray(inputs["x"])
    convoluted = np.asarray(inputs["convoluted"])

    # Kick off the big input transfer first (async, per-shard), so the
    # host-side pooling / filter-gen / gather work overlaps the wire.
    devices = r["devices"]
    cv_shards = []
    for k in range(NCORES):
        c16 = convoluted[k * SPC:(k + 1) * SPC].astype(np.float16)
        cv_shards.append(jax.device_put(
            c16.reshape(SPC, NIC, P, H, W), devices[k]))
    cv_g = jax.make_array_from_single_device_arrays(
        (B, NIC, P, H, W), r["sharding"], cv_shards)

    zeros = r["zjit"]()   # device-side; materializes while we do host work
    wdev = _device_weights(r, inputs["w_c1"], inputs["gamma"], inputs["beta"])

    # host: pooled sums / 1024 -> means; gen = wg @ pooled + bg   (exact f32)
    wg = np.asarray(inputs["w_gen"], np.float32)[:, :, 0, 0]
    bg = np.asarray(inputs["b_gen"], np.float32)
    pooled = (_pool9(x) * (1.0 / 1024.0)).astype(np.float32)   # [B, C, 9]
    genf = np.matmul(wg[None], pooled) + bg[None, :, None]
    genf = np.ascontiguousarray(genf.reshape(B, NOC, P, 9), np.float32)

    arrays = {
        "cv": cv_g,
        "genf": genf,
        "wT": wdev["wT"],
        "gam": wdev["gam"],
        "bet": wdev["bet"],
        "ident": wdev["ident"],
    }
    args = [arrays[name] for name in r["in_names"]]
    if ktime:
        print(f"[ktime] host prep + issue: {time.time() - t0:.3f}s")
    outs = r["jitted"](*args, *zeros)

    # Gather output shards in parallel; upcast straight into the f32 result.
    out = outs[0]                                   # [B, NOC, P, H, W] fp16
    if ktime:
        out.block_until_ready()
        print(f"[ktime] h2d+exec done: {time.time() - t0:.3f}s")
    shards = out.addressable_shards
    for s in shards:
        s.data.copy_to_host_async()
    res = np.empty((B, NOC, P, H, W), np.float32)
    for s in shards:
        res[s.index] = np.asarray(s.data)
    if ktime:
        print(f"[ktime] gather+upcast done: {time.time() - t0:.3f}s")
    return res.reshape(B, C, H, W)


# revision 18
# speedup vs baseline: 1.0596x; 1.0596x over previous
"""TRN2 Bass kernel for nn_DCM_50414326120808 (dense_cnn).

Computes, for x, convoluted [16, 256, 96, 96]:
  pooled = adaptive_avg_pool2d(x, 3)                         # [16,256,3,3]
  gen    = 1x1 conv (w_gen) of pooled + b_gen                # per-sample filters
  y      = conv3x3(convoluted, w_c1) + b_c1                  # [16,256,96,96]
  y      = relu(batchnorm_train(y) * gamma + beta)
  out    = depthwise 3x3 conv of y with per-(sample,channel) filters gen

Wall-clock here is dominated by the axon host<->device wire (~40-55 MB/s),
not device compute, so the design minimizes wire bytes:
 - x is never sent: pooling + the 1x1 filter generation are exact f32 host
   math (one cheap reduction pass + a tiny matmul).
 - convoluted crosses the wire in fp16 (75.5 MB instead of 151), unpadded;
   zero-padding happens on device in SBUF.
 - the output crosses back in fp16 and is upcast to f32 on host.
 - b_c1 is dropped entirely: training-mode BN subtracts the per-channel
   mean, so a constant per-channel bias cancels exactly.
 - the donated output buffers are created device-side (jnp.zeros under jit)
   instead of shipping host zeros through the tunnel.
 - a custom shard_map runner feeds one globally-sharded array per input
   (batch-sharded over 8 cores), avoiding the per-core dict -> concat copies
   of run_bass_kernel_spmd.

Device mapping (per core, 2 samples):
 - conv3x3 -> 18 accumulated TensorE matmuls (9 taps x 2 input-channel
   chunks) per 4-row output tile, fp16 operands, f32 PSUM accumulate.
 - BN stats via DVE bn_stats on each conv PSUM tile + bn_aggr + AllReduce.
 - depthwise conv -> 9 accumulated matmuls with diagonal weight matrices
   diag(gen[:, tap]) built on DVE from an identity matrix.
"""

import hashlib

import numpy as np

import jax
import jax.numpy as jnp
from jax.sharding import Mesh, NamedSharding, PartitionSpec
from jax.experimental.shard_map import shard_map

try:
    jax.config.update("jax_compilation_cache_dir", "/tmp/jax_cache_nn_dcm")
    jax.config.update("jax_persistent_cache_min_entry_size_bytes", -1)
    jax.config.update("jax_persistent_cache_min_compile_time_secs", 0.0)
except Exception:
    pass

import concourse.bass as bass  # noqa: F401  (registers bass machinery)
import concourse.bacc as bacc
import concourse.tile as tile
from concourse import mybir
from concourse.bass2jax import (
    _bass_exec_p,
    install_neuronx_cc_hook,
    partition_id_tensor,
)

F32 = mybir.dt.float32
F16 = mybir.dt.float16

B, C, H, W = 16, 256, 96, 96
FS = 3
BN_EPS = 1e-5
NCORES = 8
SPC = B // NCORES          # samples per core = 2
P = 128                    # partition dim
NIC = C // P               # input channel chunks = 2
NOC = C // P               # output channel chunks = 2
HP, WP = H + 2, W + 2      # padded spatial = 98
RT = 4                     # output rows per tile
NT = H // RT               # tiles per (sample, oc) = 24
GRP = 6                    # tiles per input group (24 rows)
NG = NT // GRP             # input groups = 4
GR = GRP * RT              # rows per input group = 24
N_LOCAL = float(SPC * H * W)        # elements per (channel, core)
N_TOTAL = float(B * H * W)          # elements per channel globally

_cache = {}


def _build_program():
    nc = bacc.Bacc("TRN2", target_bir_lowering=False, debug=False,
                   num_devices=NCORES)

    cv_d = nc.dram_tensor("cv", (SPC, NIC, P, H, W), F16, kind="ExternalInput")
    gen_d = nc.dram_tensor("genf", (SPC, NOC, P, 9), F32, kind="ExternalInput")
    wT_d = nc.dram_tensor("wT", (NIC, P, 9 * NOC * P), F16, kind="ExternalInput")
    gam_d = nc.dram_tensor("gam", (NOC, P), F32, kind="ExternalInput")
    bet_d = nc.dram_tensor("bet", (NOC, P), F32, kind="ExternalInput")
    id_d = nc.dram_tensor("ident", (P, P), F16, kind="ExternalInput")
    out_d = nc.dram_tensor("out", (SPC, NOC, P, H, W), F16, kind="ExternalOutput")

    with tile.TileContext(nc) as tc:
        with (
            tc.tile_pool(name="const", bufs=1) as const,
            tc.tile_pool(name="cin", bufs=4) as cinp,
            tc.tile_pool(name="small", bufs=1) as small,
            tc.tile_pool(name="ybn", bufs=1) as ybnp,
            tc.tile_pool(name="yld", bufs=3) as yldp,
            tc.tile_pool(name="evac", bufs=4) as evacp,
            tc.tile_pool(name="diag", bufs=2) as diagp,
            tc.tile_pool(name="ps_conv", bufs=3, space="PSUM") as ps_conv,
            tc.tile_pool(name="ps_dw", bufs=3, space="PSUM") as ps_dw,
            tc.tile_pool(name="dram", bufs=1, space="DRAM") as dram,
        ):
            # ---- constants / weights ----
            w_sb = const.tile([P, NIC, 9 * NOC * P], F16)
            for ic in range(NIC):
                nc.sync.dma_start(w_sb[:, ic, :], wT_d.ap()[ic])
            id_sb = const.tile([P, P], F16)
            nc.sync.dma_start(id_sb[:], id_d.ap())
            gam_sb = const.tile([P, NOC], F32)
            bet_sb = const.tile([P, NOC], F32)
            nc.sync.dma_start(gam_sb[:], gam_d.ap().rearrange("a p -> p a"))
            nc.sync.dma_start(bet_sb[:], bet_d.ap().rearrange("a p -> p a"))
            gen = {}
            for s in range(SPC):
                for oc in range(NOC):
                    gt = small.tile([P, 9], F32, tag=f"gen{s}{oc}",
                                    name=f"gen{s}{oc}")
                    gen[s, oc] = gt
                    nc.sync.dma_start(gt[:], gen_d.ap()[s, oc])

            y_spill = dram.tile([SPC, NOC, P, H, W], F16)
            ar_in_d = dram.tile([P, 2 * NOC], F32)
            ar_out_d = dram.tile([P, 2 * NOC], F32)

            # ---- conv3x3 (device-side zero pad) + BN stats + spill ----
            stats = small.tile([P, NOC, NT * SPC * 6], F32)
            for s in range(SPC):
                for g in range(NG):
                    cin = {}
                    for ic in range(NIC):
                        ct = cinp.tile([P, GR + 2, WP], F16, name="cin")
                        cin[ic] = ct
                        # zero the pad borders, then DMA the valid interior
                        nc.gpsimd.memset(ct[:, :, 0:1], 0)
                        nc.gpsimd.memset(ct[:, :, WP - 1:WP], 0)
                        if g == 0:
                            nc.gpsimd.memset(ct[:, 0, :], 0)
                            nc.sync.dma_start(
                                ct[:, 1:GR + 2, 1:W + 1],
                                cv_d.ap()[s, ic, :, 0:GR + 1, :])
                        elif g == NG - 1:
                            nc.gpsimd.memset(ct[:, GR + 1, :], 0)
                            nc.sync.dma_start(
                                ct[:, 0:GR + 1, 1:W + 1],
                                cv_d.ap()[s, ic, :, g * GR - 1:H, :])
                        else:
                            nc.sync.dma_start(
                                ct[:, :, 1:W + 1],
                                cv_d.ap()[s, ic, :, g * GR - 1:g * GR + GR + 1, :])
                    for jj in range(GRP):
                        j = g * GRP + jj
                        for oc in range(NOC):
                            ps = ps_conv.tile([P, RT, W], F32, name="ps")
                            k = 0
                            for ic in range(NIC):
                                for t in range(9):
                                    dy, dx = t // 3, t % 3
                                    r0 = jj * RT + dy
                                    nc.tensor.matmul(
                                        ps[:],
                                        w_sb[:, ic, (t * NOC + oc) * P:
                                             (t * NOC + oc + 1) * P],
                                        cin[ic][:, r0:r0 + RT, dx:dx + W],
                                        start=(k == 0), stop=(k == 17))
                                    k += 1
                            idx = (s * NT + j) * 6
                            nc.vector.bn_stats(
                                stats[:, oc, idx:idx + 6],
                                ps[:].rearrange("p a b -> p (a b)"))
                            ysb = evacp.tile([P, RT, W], F16, name="ysb")
                            nc.scalar.copy(ysb[:], ps[:])
                            nc.sync.dma_start(
                                y_spill[s, oc, :, j * RT:(j + 1) * RT, :], ysb[:])

            # ---- merge stats, AllReduce, compute scale/bias ----
            ar_in = small.tile([P, 2 * NOC], F32)
            mvt = small.tile([P, NOC, 2], F32)
            tmp = small.tile([P, 4], F32)
            for oc in range(NOC):
                nc.vector.bn_aggr(mvt[:, oc, :], stats[:, oc, :])
                # sum = n * mean ; sumsq = n * (var + mean^2)
                nc.vector.tensor_scalar_mul(ar_in[:, 2 * oc:2 * oc + 1],
                                            mvt[:, oc, 0:1], N_LOCAL)
                nc.vector.tensor_mul(tmp[:, 0:1], mvt[:, oc, 0:1], mvt[:, oc, 0:1])
                nc.vector.tensor_add(tmp[:, 1:2], tmp[:, 0:1], mvt[:, oc, 1:2])
                nc.vector.tensor_scalar_mul(ar_in[:, 2 * oc + 1:2 * oc + 2],
                                            tmp[:, 1:2], N_LOCAL)
            nc.sync.dma_start(ar_in_d[:], ar_in[:])
            nc.gpsimd.collective_compute(
                "AllReduce", mybir.AluOpType.add,
                replica_groups=[list(range(NCORES))],
                ins=[ar_in_d.opt()], outs=[ar_out_d.opt()])
            ar_out = small.tile([P, 2 * NOC], F32)
            nc.sync.dma_start(ar_out[:], ar_out_d[:])

            scale = small.tile([P, NOC], F32)
            bias = small.tile([P, NOC], F32)
            w1 = small.tile([P, 8], F32)
            for oc in range(NOC):
                mu = w1[:, 0:1]
                veps = w1[:, 1:2]
                nc.vector.tensor_scalar_mul(mu, ar_out[:, 2 * oc:2 * oc + 1],
                                            1.0 / N_TOTAL)
                # var = sumsq/n - mu^2 ; veps = var + eps
                nc.vector.tensor_scalar_mul(w1[:, 2:3],
                                            ar_out[:, 2 * oc + 1:2 * oc + 2],
                                            1.0 / N_TOTAL)
                nc.vector.tensor_mul(w1[:, 3:4], mu, mu)
                nc.vector.tensor_sub(w1[:, 4:5], w1[:, 2:3], w1[:, 3:4])
                nc.vector.tensor_scalar_add(veps, w1[:, 4:5], BN_EPS)
                # r = rsqrt(veps): reciprocal + ACT sqrt + one Newton step
                inv = w1[:, 5:6]
                nc.vector.reciprocal(inv, veps)
                r = w1[:, 6:7]
                nc.scalar.activation(r, inv, mybir.ActivationFunctionType.Sqrt)
                # r <- 0.5 * r * (3 - veps * r^2)
                nc.vector.tensor_mul(w1[:, 7:8], r, r)
                nc.vector.tensor_mul(w1[:, 7:8], w1[:, 7:8], veps)
                nc.vector.tensor_scalar(w1[:, 7:8], w1[:, 7:8], -0.5, 1.5,
                                        op0=mybir.AluOpType.mult,
                                        op1=mybir.AluOpType.add)
                nc.vector.tensor_mul(r, r, w1[:, 7:8])
                # scale = gamma * r ; bias = beta - mu * scale
                nc.vector.tensor_mul(scale[:, oc:oc + 1], gam_sb[:, oc:oc + 1], r)
                nc.vector.tensor_mul(w1[:, 7:8], mu, scale[:, oc:oc + 1])
                nc.vector.tensor_sub(bias[:, oc:oc + 1], bet_sb[:, oc:oc + 1],
                                     w1[:, 7:8])

            # ---- BN apply + ReLU + dynamic depthwise conv ----
            for s in range(SPC):
                for oc in range(NOC):
                    dg = diagp.tile([P, 9, P], F16, name="dg")
                    for t in range(9):
                        nc.vector.tensor_scalar_mul(dg[:, t, :], id_sb[:],
                                                    gen[s, oc][:, t:t + 1])
                    ybn = ybnp.tile([P, HP, WP], F16, name="ybn")
                    nc.gpsimd.memset(ybn[:, 0, :], 0)
                    nc.gpsimd.memset(ybn[:, HP - 1, :], 0)
                    # interior edge pads: (r, 97) and (r+1, 0) are flat-adjacent.
                    # H+1 rows so the (96, 97) corner is included — leaving it
                    # stale corrupts out[:, :, 94:96, 95] via the dx=2 taps.
                    pad_pairs = (ybn[:].rearrange("p a b -> p (a b)")
                                 [:, WP - 1:WP - 1 + (H + 1) * WP]
                                 .rearrange("p (r t) -> p r t", t=WP)[:, :, 0:2])
                    nc.gpsimd.memset(pad_pairs, 0)
                    RB = 24
                    for rb in range(H // RB):
                        yld = yldp.tile([P, RB, W], F16, name="yld")
                        nc.sync.dma_start(
                            yld[:], y_spill[s, oc, :, rb * RB:(rb + 1) * RB, :])
                        nc.scalar.activation(
                            ybn[:, 1 + rb * RB:1 + (rb + 1) * RB, 1:W + 1],
                            yld[:], mybir.ActivationFunctionType.Relu,
                            bias=bias[:, oc:oc + 1], scale=scale[:, oc:oc + 1])
                    for j in range(NT):
                        pd = ps_dw.tile([P, RT, W], F32, name="pd")
                        for t in range(9):
                            dy, dx = t // 3, t % 3
                            nc.tensor.matmul(
                                pd[:], dg[:, t, :],
                                ybn[:, j * RT + dy:j * RT + dy + RT, dx:dx + W],
                                start=(t == 0), stop=(t == 8))
                        osb = evacp.tile([P, RT, W], F16, name="osb")
                        nc.vector.tensor_copy(osb[:], pd[:])
                        nc.sync.dma_start(
                            out_d.ap()[s, oc, :, j * RT:(j + 1) * RT, :], osb[:])

    nc.compile()
    return nc


def _build_runner():
    nc = _build_program()
    install_neuronx_cc_hook()

    partition_name = (nc.partition_id_tensor.name
                      if nc.partition_id_tensor else None)
    in_names = []
    out_names = []
    out_avals = []
    for alloc in nc.m.functions[0].allocations:
        if not isinstance(alloc, mybir.MemoryLocationSet):
            continue
        name = alloc.memorylocations[0].name
        if alloc.kind == "ExternalInput":
            if name != partition_name:
                in_names.append(name)
        elif alloc.kind == "ExternalOutput":
            assert alloc.tensor_shape is not None and alloc.dtype is not None
            out_names.append(name)
            out_avals.append(jax.core.ShapedArray(
                tuple(alloc.tensor_shape), mybir.dt.np(alloc.dtype)))
    n_params = len(in_names)
    n_outs = len(out_avals)
    all_in_names = tuple(in_names) + tuple(out_names)
    if partition_name is not None:
        all_in_names = all_in_names + (partition_name,)

    def _body(*args):
        operands = list(args)
        if partition_name is not None:
            operands.append(partition_id_tensor())
        outs = _bass_exec_p.bind(
            *operands,
            out_avals=tuple(out_avals),
            in_names=all_in_names,
            out_names=tuple(out_names),
            lowering_input_output_aliases=(),
            sim_require_finite=True,
            sim_require_nnan=True,
            nc=nc,
        )
        return tuple(outs)

    devices = jax.devices()[:NCORES]
    mesh = Mesh(np.asarray(devices), ("core",))
    spec = PartitionSpec("core")
    jitted = jax.jit(
        shard_map(_body, mesh=mesh,
                  in_specs=(spec,) * (n_params + n_outs),
                  out_specs=(spec,) * n_outs,
                  check_rep=False),
        donate_argnums=tuple(range(n_params, n_params + n_outs)),
        keep_unused=True,
    )
    zero_shapes = [(NCORES * a.shape[0], *a.shape[1:]) for a in out_avals]
    zero_dtypes = [a.dtype for a in out_avals]
    zjit = jax.jit(
        lambda: tuple(jnp.zeros(s, d) for s, d in zip(zero_shapes, zero_dtypes)),
        out_shardings=tuple(NamedSharding(mesh, spec) for _ in out_avals),
    )
    # Touch every device once so connection setup doesn't land inside the
    # first real transfer.
    warm = [jax.device_put(np.zeros((8, 8), np.float16), d) for d in devices]
    for wa in warm:
        wa.block_until_ready()

    return {"nc": nc, "jitted": jitted, "zjit": zjit,
            "in_names": in_names, "out_names": out_names,
            "mesh": mesh, "devices": devices,
            "sharding": NamedSharding(mesh, spec)}


def _pool9(x):
    """Exact adaptive_avg_pool2d(x, 3) sums, flattened: [B, C, 9] (f32 sums)."""
    x = np.asarray(x, np.float32)
    r = x.reshape(B * C, H, 3, W // 3).sum(axis=3)
    p = r.reshape(B * C, 3, H // 3, 3).sum(axis=2)
    return p.reshape(B, C, 9)


def _device_weights(r, w_c1, gamma, beta):
    """Transfer the (replicated) conv/BN params once; reuse across calls."""
    w_c1 = np.asarray(w_c1, np.float32)
    gamma = np.asarray(gamma, np.float32)
    beta = np.asarray(beta, np.float32)
    key = hashlib.blake2b(
        w_c1.tobytes() + gamma.tobytes() + beta.tobytes(), digest_size=16
    ).digest()
    if _cache.get("wkey") == key:
        return _cache["wdev"]
    # wT[ic, i, ((t*NOC)+oc)*P+o] = w_c1[oc*P+o, ic*P+i, dy, dx]
    wT = np.ascontiguousarray(
        w_c1.reshape(NOC, P, NIC, P, 9).transpose(2, 3, 4, 0, 1)
    ).reshape(NIC, P, 9 * NOC * P).astype(np.float16)
    sh = r["sharding"]
    wdev = {
        "wT": jax.device_put(np.tile(wT, (NCORES, 1, 1)), sh),
        "gam": jax.device_put(np.tile(gamma.reshape(NOC, P), (NCORES, 1)), sh),
        "bet": jax.device_put(np.tile(beta.reshape(NOC, P), (NCORES, 1)), sh),
        "ident": jax.device_put(
            np.tile(np.eye(P, dtype=np.float16), (NCORES, 1)), sh),
    }
    _cache["wkey"] = key
    _cache["wdev"] = wdev
    return wdev


def kernel(**inputs) -> np.ndarray:
    import os
    import time
    ktime = os.environ.get("KTIME", "0") == "1"
    t0 = time.time()

    if "runner" not in _cache:
        _cache["runner"] = _build_runner()
    r = _cache["runner"]

    x = np.asarray(inputs["x"])
    convoluted = np.asarray(inputs["convoluted"])

    # Kick off the big input transfer first (async, per-shard), so the
    # host-side pooling / filter-gen / gather work overlaps the wire.
    devices = r["devices"]
    cv_shards = []
    for k in range(NCORES):
        c16 = convoluted[k * SPC:(k + 1) * SPC].astype(np.float16)
        cv_shards.append(jax.device_put(
            c16.reshape(SPC, NIC, P, H, W), devices[k]))
    cv_g = jax.make_array_from_single_device_arrays(
        (B, NIC, P, H, W), r["sharding"], cv_shards)

    zeros = r["zjit"]()   # device-side; materializes while we do host work
    wdev = _device_weights(r, inputs["w_c1"], inputs["gamma"], inputs["beta"])

    # host: pooled sums / 1024 -> means; gen = wg @ pooled + bg   (exact f32)
    wg = np.asarray(inputs["w_gen"], np.float32)[:, :, 0, 0]
    bg = np.asarray(inputs["b_gen"], np.float32)
    pooled = (_pool9(x) * (1.0 / 1024.0)).astype(np.float32)   # [B, C, 9]
    genf = np.matmul(wg[None], pooled) + bg[None, :, None]
    genf = np.ascontiguousarray(genf.reshape(B, NOC, P, 9), np.float32)

    arrays = {
        "cv": cv_g,
        "genf": genf,
        "wT": wdev["wT"],
        "gam": wdev["gam"],
        "bet": wdev["bet"],
        "ident": wdev["ident"],
    }
    args = [arrays[name] for name in r["in_names"]]
    if ktime:
        print(f"[ktime] host prep + issue: {time.time() - t0:.3f}s")
    outs = r["jitted"](*args, *zeros)

    # Gather output shards in parallel; upcast straight into the f32 result.
    out = outs[0]                                   # [B, NOC, P, H, W] fp16
    if ktime:
        out.block_until_ready()
        print(f"[ktime] h2d+exec done: {time.time() - t0:.3f}s")
    shards = out.addressable_shards
    for s in shards:
        s.data.copy_to_host_async()
    res = np.empty((B, NOC, P, H, W), np.float32)
    for s in shards:
        res[s.index] = np.asarray(s.data)
    if ktime:
        print(f"[ktime] gather+upcast done: {time.time() - t0:.3f}s")
    return res.reshape(B, C, H, W)
